# revision 11
# baseline (speedup 1.0000x reference)
"""Chebyshev graph-conv kernel for Trainium2 (8 NeuronCores, SPMD).

Math: out[b,o,m,t] = sum_{k,c,n} T[k,n,m] * x[b,c,n,t] * Theta[k,c,o]
with T the Chebyshev polynomials of the normalized adjacency (n=24, K=3).

The whole operator collapses into a single 768x768 matrix
    W[(c,n),(o,m)] = sum_k Theta[k,c,o] * T[k,n,m]
applied per batch element to x[b] viewed as (c*n, t) = (768, 512):
    out[b](o*24+m, t) = W.T-contract over rows -> exactly one matmul chain.

W is tiny and computed on host from adj/Theta; x is read once and out
written once. Data-parallel over batch: 64 -> 8 per core. All device I/O is
fp16 (PE multiplies 16-bit at full rate with hidden weight loads; fp16 I/O
halves HBM traffic both ways). PSUM accumulation is fp32; the final fp32
cast happens on host (fp16 store rounding costs ~2e-4 extra relative
error). Per core: 8 batch elements, each a 6x6 chain of [128,128]x[128,512]
matmuls.

Schedule notes (from NTFF traces): the bass framework preamble costs a
fixed ~7.2us before any kernel instruction, and the PE-bound matmul stream
(288 MMs @ 215.5ns warm) is the roofline at ~62us. So the kernel is
structured to start real matmuls as early as possible after the preamble:

- i-outer loop order: all 6 output-chunk PSUM banks accumulate in parallel
  per contraction chunk i, so the first matmul needs only W[0]/x0[0]
  (0.3 MB) instead of the whole first batch (1.95 MB).
- W chunks go on the Sync HWDGE ring while x0 chunks go concurrently on
  the Scalar ring (stores don't exist yet), halving time-to-first-chunk.
- HAM warm-up: the PE boots clock-gated at K=4/8 (1.2 GHz) and unthrottles
  only after ~3.4us of sustained busy. A vector-engine memset (no slow
  gpsimd dependency) plus a few dummy matmuls keep the PE busy from the
  first possible instant so the unthrottle deadline starts ticking ~7.4us
  rather than when DMA data lands.
"""

import numpy as np

import concourse.mybir as mybir
from concourse import bacc, tile
from concourse.bass import _add_dep_helper
from concourse.bass_utils import run_bass_kernel_spmd

N_CORES = 8
B, C, NV, T = 64, 32, 24, 512
K = 3
O = 32
CN = C * NV   # 768 contraction rows
OM = O * NV   # 768 output rows
BP = B // N_CORES  # 8 batch elements per core
P = 128
NBLK = CN // P  # 6

_compiled_nc = None
last_result = None  # BassKernelResults from the most recent run (for test.py)


def _build_nc():
    f32 = mybir.dt.float32
    f16 = mybir.dt.float16
    nc = bacc.Bacc("TRN2", target_bir_lowering=False, debug=False,
                   num_devices=N_CORES)
    xs = nc.dram_tensor("xs", [BP, CN, T], f16, kind="ExternalInput")
    w = nc.dram_tensor("w", [CN, OM], f16, kind="ExternalInput")
    out = nc.dram_tensor("out", [BP, OM, T], f16, kind="ExternalOutput")

    wr = w[:].rearrange("(i p) m -> p i m", p=P)

    with tile.TileContext(nc) as tc:
        with (
            tc.tile_pool(name="wpool", bufs=1) as wpool,
            tc.tile_pool(name="xpool", bufs=8) as xpool,
            tc.tile_pool(name="opool", bufs=6) as opool,
            tc.tile_pool(name="psum", bufs=8, space="PSUM") as psum_pool,
        ):
            # HAM warm-up. Trace-measured head physics: the first kernel
            # instruction runs at ~7.2us (framework preamble), HWDGE
            # first-byte latency is 1.5-2.3us, and DMA completion adds
            # ~0.5-1us receipt latency, so the first real operands cannot
            # be consumable before ~10.2us. Dummy matmuls must keep the PE
            # busy from ~7.7us until then: an idle gap restarts the HAM
            # 3.4us sustained-busy clock and costs ~10 extra cold matmuls
            # (measured). 13 x 256-col cold MMs (~214ns each) bridge it.
            warm = wpool.tile([P, 256], f16, tag="warm")
            nc.vector.memset(warm[:], 0.0)
            for wi in range(12):
                wps = psum_pool.tile([P, T], f32, name=f"wps{wi}", tag="psr")
                nc.tensor.matmul(wps[:, :256], warm[:, :P], warm[:],
                                 start=True, stop=True)

            # Loads must be progressive (one DMA = one completion sem, so a
            # bulk transfer's data is unusable until ALL of it lands), but
            # the SDMA engines round-robin among ALL queued transfers at
            # packet granularity, so concurrently-queued DMAs all finish
            # together. To make the critical first chunks land first, only
            # they are queued at t=0 (x0[0] + W[0,j=0] on Sync, rest of
            # W[0] on Scalar); every other load is chained behind its
            # predecessor's completion with an explicit dep.
            wt = wpool.tile([P, NBLK, OM], f16)
            xt0 = xpool.tile([P, NBLK, T], f16)
            xr0 = xs[0].rearrange("(i p) t -> p i t", p=P)
            d_prev_x = nc.sync.dma_start(xt0[:, 0, :], xr0[:, 0, :])
            nc.sync.dma_start(wt[:, 0, 0:P], wr[:, 0, 0:P])
            d_prev_w = nc.scalar.dma_start(wt[:, 0, P:], wr[:, 0, P:])
            for i in range(1, NBLK):
                d = nc.sync.dma_start(xt0[:, i, :], xr0[:, i, :])
                _add_dep_helper(d.ins, d_prev_x.ins, sync=True,
                                reason="serialize x0 chunks in need-order")
                d_prev_x = d
                d = nc.scalar.dma_start(wt[:, i, :], wr[:, i, :])
                _add_dep_helper(d.ins, d_prev_w.ins, sync=True,
                                reason="serialize W chunks in need-order")
                d_prev_w = d

            xts = [xt0]
            for b in range(1, BP):
                xt = xpool.tile([P, NBLK, T], f16, tag="xt0")
                xr = xs[b].rearrange("(i p) t -> p i t", p=P)
                d = nc.sync.dma_start(xt[:], xr)
                _add_dep_helper(d.ins, d_prev_x.ins, sync=True,
                                reason="serialize batch loads in need-order")
                d_prev_x = d
                xts.append(xt)

            for b in range(BP):
                xt = xts[b]
                ot = opool.tile([P, NBLK, T], f16)
                orr = out[b].rearrange("(j p) t -> p j t", p=P)
                if b == 0:
                    # i-outer: all 6 j-banks accumulate in parallel so
                    # contraction chunk i is consumed by 6 back-to-back
                    # matmuls as soon as it lands; the first matmul needs
                    # only W[0,0:128]/x0[0], not the whole batch.
                    pss = [psum_pool.tile([P, T], f32, name=f"ps0_j{j}",
                                          tag="psr")
                           for j in range(NBLK)]
                    for i in range(NBLK):
                        for j in range(NBLK):
                            nc.tensor.matmul(
                                pss[j][:],
                                wt[:, i, j * P:(j + 1) * P],
                                xt[:, i, :],
                                start=(i == 0),
                                stop=(i == NBLK - 1),
                            )
                    for j in range(NBLK):
                        nc.vector.tensor_copy(ot[:, j, :], pss[j][:])
                else:
                    # j-outer: x is fully resident by now; copies spread
                    # across the batch so the PSUM->SBUF->HBM pipeline never
                    # bunches at a batch boundary (matters for the tail).
                    for j in range(NBLK):
                        ps = psum_pool.tile([P, T], f32,
                                            name=f"ps_b{b}_j{j}", tag="psr")
                        for i in range(NBLK):
                            nc.tensor.matmul(
                                ps[:],
                                wt[:, i, j * P:(j + 1) * P],
                                xt[:, i, :],
                                start=(i == 0),
                                stop=(i == NBLK - 1),
                            )
                        # fp32 PSUM -> fp16 SBUF cast (16-bit DVE rate)
                        nc.vector.tensor_copy(ot[:, j, :], ps[:])
                        if b == BP - 1:
                            # Last batch: per-j stores right after each
                            # copy, alternating across both now-idle rings,
                            # so the store left after the final copy is
                            # only 128 KB.
                            eng = nc.sync if j % 2 == 0 else nc.scalar
                            eng.dma_start(orr[:, j, :], ot[:, j, :])
                if b < BP - 1:
                    # One whole-batch store (1 issue, 768 KB) on the Scalar
                    # ring once all 6 copies are done; production-paced.
                    nc.scalar.dma_start(orr[:], ot[:])

    nc.compile()
    return nc


def _combined_operator(adj: np.ndarray, Theta: np.ndarray) -> np.ndarray:
    """W[(c,n),(o,m)] = sum_k Theta[k,c,o] * T[k,n,m], fp16, shape (768,768)."""
    adj = np.asarray(adj).astype(np.float32)
    Theta = np.asarray(Theta)
    d = adj.sum(axis=1)
    d_inv_sqrt = np.where(d > 0, 1.0 / np.sqrt(d), 0.0).astype(np.float32)
    L = (adj * d_inv_sqrt[None, :]).T * d_inv_sqrt[None, :]
    Ts = [np.eye(NV, dtype=np.float32), L.astype(np.float32)]
    for _ in range(2, K):
        Ts.append((2.0 * L @ Ts[-1] - Ts[-2]).astype(np.float32))
    Tcheb = np.stack(Ts[:K])  # (K, n, m)
    W = np.einsum("kco,knm->cnom", Theta.astype(np.float32), Tcheb)
    return np.ascontiguousarray(W.reshape(CN, OM), dtype=np.float16)


def kernel(x: np.ndarray, adj: np.ndarray, Theta: np.ndarray) -> np.ndarray:
    global _compiled_nc, last_result
    if _compiled_nc is None:
        _compiled_nc = _build_nc()
    nc = _compiled_nc

    W = _combined_operator(adj, Theta)
    # x: (64, 32, 24, 512) -> per-core shard [8, 768, 512], fp16 (the device
    # matmul consumes fp16 regardless; casting host-side halves HBM reads)
    xf = np.asarray(x).astype(np.float16).reshape(B, CN, T)
    in_maps = [
        {"xs": np.ascontiguousarray(xf[c * BP:(c + 1) * BP]), "w": W}
        for c in range(N_CORES)
    ]
    res = run_bass_kernel_spmd(nc, in_maps, core_ids=list(range(N_CORES)))
    last_result = res
    out = np.concatenate([r["out"] for r in res.results], axis=0)
    return np.ascontiguousarray(out.reshape(B, O, NV, T).astype(np.float32))


# revision 12
# speedup vs baseline: 1.0966x; 1.0966x over previous
"""Chebyshev graph-conv kernel for Trainium2 (8 NeuronCores, SPMD).

Math: out[b,o,m,t] = sum_{k,c,n} T[k,n,m] * x[b,c,n,t] * Theta[k,c,o]
with T the Chebyshev polynomials of the normalized adjacency (n=24, K=3).

The whole operator collapses into a single 768x768 matrix
    W[(c,n),(o,m)] = sum_k Theta[k,c,o] * T[k,n,m]
applied per batch element to x[b] viewed as (c*n, t) = (768, 512):
    out[b](o*24+m, t) = W.T-contract over rows -> exactly one matmul chain.

W is tiny and computed on host from adj/Theta; x is read once and out
written once. Data-parallel over batch: 64 -> 8 per core. All device I/O is
fp16 (PE multiplies 16-bit at full rate with hidden weight loads; fp16 I/O
halves HBM traffic both ways). PSUM accumulation is fp32; the final fp32
cast happens on host (fp16 store rounding costs ~2e-4 extra relative
error). Per core: 8 batch elements, each a 6x6 chain of [128,128]x[128,512]
matmuls.

Schedule notes (from NTFF traces): the bass framework preamble costs a
fixed ~7.2us before any kernel instruction, and the PE-bound matmul stream
(288 MMs @ 215.5ns warm) is the roofline at ~62us. So the kernel is
structured to start real matmuls as early as possible after the preamble:

- i-outer loop order: all 6 output-chunk PSUM banks accumulate in parallel
  per contraction chunk i, so the first matmul needs only W[0]/x0[0]
  (0.3 MB) instead of the whole first batch (1.95 MB).
- W chunks go on the Sync HWDGE ring while x0 chunks go concurrently on
  the Scalar ring (stores don't exist yet), halving time-to-first-chunk.
- HAM warm-up: the PE boots clock-gated at K=4/8 (1.2 GHz) and unthrottles
  only after ~3.4us of sustained busy. A vector-engine memset (no slow
  gpsimd dependency) plus a few dummy matmuls keep the PE busy from the
  first possible instant so the unthrottle deadline starts ticking ~7.4us
  rather than when DMA data lands.
"""

import numpy as np

import concourse.mybir as mybir
from concourse import bacc, tile
from concourse.bass import _add_dep_helper
from concourse.bass_utils import run_bass_kernel_spmd

N_CORES = 8
B, C, NV, T = 64, 32, 24, 512
K = 3
O = 32
CN = C * NV   # 768 contraction rows
OM = O * NV   # 768 output rows
BP = B // N_CORES  # 8 batch elements per core
P = 128
NBLK = CN // P  # 6

_compiled_nc = None
last_result = None  # BassKernelResults from the most recent run (for test.py)


def _build_nc():
    f32 = mybir.dt.float32
    f16 = mybir.dt.float16
    nc = bacc.Bacc("TRN2", target_bir_lowering=False, debug=False,
                   num_devices=N_CORES)
    xs = nc.dram_tensor("xs", [BP, CN, T], f16, kind="ExternalInput")
    w = nc.dram_tensor("w", [CN, OM], f16, kind="ExternalInput")
    out = nc.dram_tensor("out", [BP, OM, T], f16, kind="ExternalOutput")

    wr = w[:].rearrange("(i p) m -> p i m", p=P)

    with tile.TileContext(nc) as tc:
        with (
            tc.tile_pool(name="wpool", bufs=1) as wpool,
            tc.tile_pool(name="xpool", bufs=8) as xpool,
            tc.tile_pool(name="opool", bufs=6) as opool,
            tc.tile_pool(name="psum", bufs=8, space="PSUM") as psum_pool,
        ):
            # HAM warm-up. Trace-measured head physics: the first kernel
            # instruction runs at ~7.2us (framework preamble), HWDGE
            # first-byte latency is 1.5-2.3us, and DMA completion adds
            # ~0.5-1us receipt latency, so the first real operands cannot
            # be consumable before ~10.2us. Dummy matmuls must keep the PE
            # busy from ~7.7us until then: an idle gap restarts the HAM
            # 3.4us sustained-busy clock and costs ~10 extra cold matmuls
            # (measured). 13 x 256-col cold MMs (~214ns each) bridge it.
            # HAM warm-up sizing: SDMA engines round-robin among all queued
            # transfers at packet granularity and run only ~175GB/s for the
            # first ~2us, so the head working set (W + x0, 1.95MB, queued
            # per-chunk with no serialization - explicit dep-chains add
            # ~1.4us dead time per link and starve the PE, measured) lands
            # together around ~10.3-11us. Warm-ups must bridge the PE from
            # ~7.3us to that moment with NO idle gap: even a 0.45us gap
            # measurably restarts the HAM sustained-busy clock and costs
            # +1.5-5us of extra cold matmuls.
            warm = wpool.tile([P, 256], f16, tag="warm")
            nc.vector.memset(warm[:], 0.0)
            for wi in range(16):
                wps = psum_pool.tile([P, T], f32, name=f"wps{wi}", tag="psr")
                nc.tensor.matmul(wps[:, :256], warm[:, :P], warm[:],
                                 start=True, stop=True)

            wt = wpool.tile([P, NBLK, OM], f16)
            xt0 = xpool.tile([P, NBLK, T], f16)
            xr0 = xs[0].rearrange("(i p) t -> p i t", p=P)
            for i in range(NBLK):
                nc.sync.dma_start(wt[:, i, :], wr[:, i, :])
                nc.scalar.dma_start(xt0[:, i, :], xr0[:, i, :])

            xts = [xt0]
            for b in range(1, BP):
                xt = xpool.tile([P, NBLK, T], f16, tag="xt0")
                xr = xs[b].rearrange("(i p) t -> p i t", p=P)
                nc.sync.dma_start(xt[:], xr)
                xts.append(xt)

            for b in range(BP):
                xt = xts[b]
                ot = opool.tile([P, NBLK, T], f16)
                orr = out[b].rearrange("(j p) t -> p j t", p=P)
                if b == 0:
                    # i-outer: all 6 j-banks accumulate in parallel so
                    # contraction chunk i is consumed by 6 back-to-back
                    # matmuls as soon as it lands; the first matmul needs
                    # only W[0,0:128]/x0[0], not the whole batch.
                    pss = [psum_pool.tile([P, T], f32, name=f"ps0_j{j}",
                                          tag="psr")
                           for j in range(NBLK)]
                    for i in range(NBLK):
                        for j in range(NBLK):
                            nc.tensor.matmul(
                                pss[j][:],
                                wt[:, i, j * P:(j + 1) * P],
                                xt[:, i, :],
                                start=(i == 0),
                                stop=(i == NBLK - 1),
                            )
                    for j in range(NBLK):
                        nc.vector.tensor_copy(ot[:, j, :], pss[j][:])
                else:
                    # j-outer: x is fully resident by now; copies spread
                    # across the batch so the PSUM->SBUF->HBM pipeline never
                    # bunches at a batch boundary (matters for the tail).
                    for j in range(NBLK):
                        ps = psum_pool.tile([P, T], f32,
                                            name=f"ps_b{b}_j{j}", tag="psr")
                        for i in range(NBLK):
                            nc.tensor.matmul(
                                ps[:],
                                wt[:, i, j * P:(j + 1) * P],
                                xt[:, i, :],
                                start=(i == 0),
                                stop=(i == NBLK - 1),
                            )
                        # fp32 PSUM -> fp16 SBUF cast (16-bit DVE rate)
                        nc.vector.tensor_copy(ot[:, j, :], ps[:])
                        if b == BP - 1:
                            # Last batch: per-j stores right after each
                            # copy, alternating across both now-idle rings,
                            # so the store left after the final copy is
                            # only 128 KB.
                            eng = nc.sync if j % 2 == 0 else nc.scalar
                            eng.dma_start(orr[:, j, :], ot[:, j, :])
                if b < BP - 1:
                    # One whole-batch store (1 issue, 768 KB) on the Scalar
                    # ring once all 6 copies are done; production-paced.
                    nc.scalar.dma_start(orr[:], ot[:])

    nc.compile()
    return nc


def _combined_operator(adj: np.ndarray, Theta: np.ndarray) -> np.ndarray:
    """W[(c,n),(o,m)] = sum_k Theta[k,c,o] * T[k,n,m], fp16, shape (768,768)."""
    adj = np.asarray(adj).astype(np.float32)
    Theta = np.asarray(Theta)
    d = adj.sum(axis=1)
    d_inv_sqrt = np.where(d > 0, 1.0 / np.sqrt(d), 0.0).astype(np.float32)
    L = (adj * d_inv_sqrt[None, :]).T * d_inv_sqrt[None, :]
    Ts = [np.eye(NV, dtype=np.float32), L.astype(np.float32)]
    for _ in range(2, K):
        Ts.append((2.0 * L @ Ts[-1] - Ts[-2]).astype(np.float32))
    Tcheb = np.stack(Ts[:K])  # (K, n, m)
    W = np.einsum("kco,knm->cnom", Theta.astype(np.float32), Tcheb)
    return np.ascontiguousarray(W.reshape(CN, OM), dtype=np.float16)


def kernel(x: np.ndarray, adj: np.ndarray, Theta: np.ndarray) -> np.ndarray:
    global _compiled_nc, last_result
    if _compiled_nc is None:
        _compiled_nc = _build_nc()
    nc = _compiled_nc

    W = _combined_operator(adj, Theta)
    # x: (64, 32, 24, 512) -> per-core shard [8, 768, 512], fp16 (the device
    # matmul consumes fp16 regardless; casting host-side halves HBM reads)
    xf = np.asarray(x).astype(np.float16).reshape(B, CN, T)
    in_maps = [
        {"xs": np.ascontiguousarray(xf[c * BP:(c + 1) * BP]), "w": W}
        for c in range(N_CORES)
    ]
    res = run_bass_kernel_spmd(nc, in_maps, core_ids=list(range(N_CORES)))
    last_result = res
    out = np.concatenate([r["out"] for r in res.results], axis=0)
    return np.ascontiguousarray(out.reshape(B, O, NV, T).astype(np.float32))


# revision 14
# speedup vs baseline: 1.1347x; 1.0348x over previous
"""Chebyshev graph-conv kernel for Trainium2 (8 NeuronCores, SPMD).

Math: out[b,o,m,t] = sum_{k,c,n} T[k,n,m] * x[b,c,n,t] * Theta[k,c,o]
with T the Chebyshev polynomials of the normalized adjacency (n=24, K=3).

The whole operator collapses into a single 768x768 matrix
    W[(c,n),(o,m)] = sum_k Theta[k,c,o] * T[k,n,m]
applied per batch element to x[b] viewed as (c*n, t) = (768, 512):
    out[b](o*24+m, t) = W.T-contract over rows -> exactly one matmul chain.

W is tiny and computed on host from adj/Theta; x is read once and out
written once. Data-parallel over batch: 64 -> 8 per core. All device I/O is
fp16 (PE multiplies 16-bit at full rate with hidden weight loads; fp16 I/O
halves HBM traffic both ways). PSUM accumulation is fp32; the final fp32
cast happens on host (fp16 store rounding costs ~2e-4 extra relative
error). Per core: 8 batch elements, each a 6x6 chain of [128,128]x[128,512]
matmuls.

Schedule notes (from NTFF traces): the bass framework preamble costs a
fixed ~7.2us before any kernel instruction, and the PE-bound matmul stream
(288 MMs @ 215.5ns warm) is the roofline at ~62us. So the kernel is
structured to start real matmuls as early as possible after the preamble:

- i-outer loop order: all 6 output-chunk PSUM banks accumulate in parallel
  per contraction chunk i, so the first matmul needs only W[0]/x0[0]
  (0.3 MB) instead of the whole first batch (1.95 MB).
- W chunks go on the Sync HWDGE ring while x0 chunks go concurrently on
  the Scalar ring (stores don't exist yet), halving time-to-first-chunk.
- HAM warm-up: the PE boots clock-gated at K=4/8 (1.2 GHz) and unthrottles
  only after ~3.4us of sustained busy. A vector-engine memset (no slow
  gpsimd dependency) plus a few dummy matmuls keep the PE busy from the
  first possible instant so the unthrottle deadline starts ticking ~7.4us
  rather than when DMA data lands.
"""

import numpy as np

import concourse.mybir as mybir
from concourse import bacc, tile
from concourse.bass import _add_dep_helper
from concourse.bass_utils import run_bass_kernel_spmd

N_CORES = 8
B, C, NV, T = 64, 32, 24, 512
K = 3
O = 32
CN = C * NV   # 768 contraction rows
OM = O * NV   # 768 output rows
BP = B // N_CORES  # 8 batch elements per core
P = 128
NBLK = CN // P  # 6

_compiled_nc = None
last_result = None  # BassKernelResults from the most recent run (for test.py)


def _build_nc():
    f32 = mybir.dt.float32
    f16 = mybir.dt.float16
    nc = bacc.Bacc("TRN2", target_bir_lowering=False, debug=False,
                   num_devices=N_CORES)
    xs = nc.dram_tensor("xs", [BP, CN, T], f16, kind="ExternalInput")
    w = nc.dram_tensor("w", [CN, OM], f16, kind="ExternalInput")
    out = nc.dram_tensor("out", [BP, OM, T], f16, kind="ExternalOutput")

    wr = w[:].rearrange("(i p) m -> p i m", p=P)

    with tile.TileContext(nc) as tc:
        with (
            tc.tile_pool(name="wpool", bufs=1) as wpool,
            tc.tile_pool(name="xpool", bufs=3) as xpool,
            tc.tile_pool(name="opool", bufs=6) as opool,
            tc.tile_pool(name="psum", bufs=8, space="PSUM") as psum_pool,
        ):
            # HAM warm-up. Trace-measured head physics: the first kernel
            # instruction runs at ~7.2us (framework preamble), HWDGE
            # first-byte latency is 1.5-2.3us, and DMA completion adds
            # ~0.5-1us receipt latency, so the first real operands cannot
            # be consumable before ~10.2us. Dummy matmuls must keep the PE
            # busy from ~7.7us until then: an idle gap restarts the HAM
            # 3.4us sustained-busy clock and costs ~10 extra cold matmuls
            # (measured). 13 x 256-col cold MMs (~214ns each) bridge it.
            # HAM warm-up sizing: SDMA engines round-robin among all queued
            # transfers at packet granularity and run only ~175GB/s for the
            # first ~2us, so the head working set (W + x0, 1.95MB, queued
            # per-chunk with no serialization - explicit dep-chains add
            # ~1.4us dead time per link and starve the PE, measured) lands
            # together around ~10.3-11us. Warm-ups must bridge the PE from
            # ~7.3us to that moment with NO idle gap: even a 0.45us gap
            # measurably restarts the HAM sustained-busy clock and costs
            # +1.5-5us of extra cold matmuls.
            # Measured: W[0] lands ~9.9us, x0[0] ~10.3us (per-ring delivery
            # staggers in issue order, ~0.85us/chunk after that). Bridge
            # 7.3-7.7 -> ~10.3 with 12x256-col cold MMs (~214ns each), then
            # 6x128-col (~107ns) so the warmup->real junction is fine-
            # grained: overshoot costs <=107ns/step, undershoot (PE idle)
            # restarts the HAM clock and costs 1.5-5us.
            warm = wpool.tile([P, 256], f16, tag="warm")
            nc.vector.memset(warm[:], 0.0)
            for wi in range(18):
                wps = psum_pool.tile([P, T], f32, name=f"wps{wi}", tag="psr")
                cols = 256 if wi < 12 else P
                nc.tensor.matmul(wps[:, :cols], warm[:, :P], warm[:, :cols],
                                 start=True, stop=True)

            wt = wpool.tile([P, NBLK, OM], f16)
            xt0 = xpool.tile([P, NBLK, T], f16)
            xr0 = xs[0].rearrange("(i p) t -> p i t", p=P)
            for i in range(NBLK):
                nc.sync.dma_start(wt[:, i, :], wr[:, i, :])
                nc.scalar.dma_start(xt0[:, i, :], xr0[:, i, :])

            xts = [xt0]
            for b in range(1, BP):
                xt = xpool.tile([P, NBLK, T], f16, tag="xt0")
                xr = xs[b].rearrange("(i p) t -> p i t", p=P)
                nc.sync.dma_start(xt[:], xr)
                xts.append(xt)

            for b in range(BP):
                xt = xts[b]
                ot = opool.tile([P, NBLK, T], f16)
                orr = out[b].rearrange("(j p) t -> p j t", p=P)
                if b == 0:
                    # i-outer: all 6 j-banks accumulate in parallel so
                    # contraction chunk i is consumed by 6 back-to-back
                    # matmuls as soon as it lands; the first matmul needs
                    # only W[0,0:128]/x0[0], not the whole batch.
                    pss = [psum_pool.tile([P, T], f32, name=f"ps0_j{j}",
                                          tag="psr")
                           for j in range(NBLK)]
                    for i in range(NBLK):
                        for j in range(NBLK):
                            nc.tensor.matmul(
                                pss[j][:],
                                wt[:, i, j * P:(j + 1) * P],
                                xt[:, i, :],
                                start=(i == 0),
                                stop=(i == NBLK - 1),
                            )
                    for j in range(NBLK):
                        nc.vector.tensor_copy(ot[:, j, :], pss[j][:])
                else:
                    # j-outer: x is fully resident by now; copies spread
                    # across the batch so the PSUM->SBUF->HBM pipeline never
                    # bunches at a batch boundary (matters for the tail).
                    for j in range(NBLK):
                        ps = psum_pool.tile([P, T], f32,
                                            name=f"ps_b{b}_j{j}", tag="psr")
                        for i in range(NBLK):
                            nc.tensor.matmul(
                                ps[:],
                                wt[:, i, j * P:(j + 1) * P],
                                xt[:, i, :],
                                start=(i == 0),
                                stop=(i == NBLK - 1),
                            )
                        # fp32 PSUM -> fp16 SBUF cast (16-bit DVE rate)
                        nc.vector.tensor_copy(ot[:, j, :], ps[:])
                        if b == BP - 1:
                            # Last batch: per-j stores right after each
                            # copy, alternating across both now-idle rings,
                            # so the store left after the final copy is
                            # only 128 KB.
                            eng = nc.sync if j % 2 == 0 else nc.scalar
                            eng.dma_start(orr[:, j, :], ot[:, j, :])
                if b < BP - 1:
                    # One whole-batch store (1 issue, 768 KB) on the Scalar
                    # ring once all 6 copies are done; production-paced.
                    nc.scalar.dma_start(orr[:], ot[:])

    nc.compile()
    return nc


def _combined_operator(adj: np.ndarray, Theta: np.ndarray) -> np.ndarray:
    """W[(c,n),(o,m)] = sum_k Theta[k,c,o] * T[k,n,m], fp16, shape (768,768)."""
    adj = np.asarray(adj).astype(np.float32)
    Theta = np.asarray(Theta)
    d = adj.sum(axis=1)
    d_inv_sqrt = np.where(d > 0, 1.0 / np.sqrt(d), 0.0).astype(np.float32)
    L = (adj * d_inv_sqrt[None, :]).T * d_inv_sqrt[None, :]
    Ts = [np.eye(NV, dtype=np.float32), L.astype(np.float32)]
    for _ in range(2, K):
        Ts.append((2.0 * L @ Ts[-1] - Ts[-2]).astype(np.float32))
    Tcheb = np.stack(Ts[:K])  # (K, n, m)
    W = np.einsum("kco,knm->cnom", Theta.astype(np.float32), Tcheb)
    return np.ascontiguousarray(W.reshape(CN, OM), dtype=np.float16)


def kernel(x: np.ndarray, adj: np.ndarray, Theta: np.ndarray) -> np.ndarray:
    global _compiled_nc, last_result
    if _compiled_nc is None:
        _compiled_nc = _build_nc()
    nc = _compiled_nc

    W = _combined_operator(adj, Theta)
    # x: (64, 32, 24, 512) -> per-core shard [8, 768, 512], fp16 (the device
    # matmul consumes fp16 regardless; casting host-side halves HBM reads)
    xf = np.asarray(x).astype(np.float16).reshape(B, CN, T)
    in_maps = [
        {"xs": np.ascontiguousarray(xf[c * BP:(c + 1) * BP]), "w": W}
        for c in range(N_CORES)
    ]
    res = run_bass_kernel_spmd(nc, in_maps, core_ids=list(range(N_CORES)))
    last_result = res
    out = np.concatenate([r["out"] for r in res.results], axis=0)
    return np.ascontiguousarray(out.reshape(B, O, NV, T).astype(np.float32))
